# revision 1
# baseline (speedup 1.0000x reference)
"""Bass/Trainium2 kernel for nn_BayesianResNet_71408126263673.

Grouped per-sample conv: for each of 32 samples i,
  out[i] = conv2d(x[i] [128,32,32], W[i] [128oc,128c,3,3], pad=1, stride=1) + bias[i]

Sharding: b_i (32 samples) split across 8 NeuronCores, 4 samples per core.
Pure data parallel, no collectives.

Per-core kernel: each sample's conv is computed as 9 accumulating matmuls
(one per 3x3 tap) into PSUM:
  out[oc, pix] = sum_{kh,kw} W[:, :, kh, kw].T @ xpad[:, shifted pix]
with K=c=128 (partition/contraction), M=oc=128, N=512 pixels (16 output rows
per PSUM bank). The input image is zero-padded to 34x34 on the HOST so DMA
loads are fully contiguous and no memset/masking is needed on-chip. Weights
are pre-transposed on the host to [c, kh*kw, oc] so each tap is a ready-to-use
lhsT (stationary operand) tile; the per-sample bias rides along as two extra
columns holding its fp32 bit pattern (bias[oc] on partition oc, read back
via bitcast), eliminating a separate tiny-packet bias DMA (measured ~1.3us
of queue time for 2KB in 16B packets).

Measured DMA facts driving the schedule (v4): one SP-HWDGE queue saturates
HBM reads at ~0.25 B/ns (a second queue adds almost nothing, and the ACT
queue only manages ~0.1 B/ns on stores); each DMA_DIRECT2D issue occupies
the issuing engine ~0.65us; per-DMA completion latency is ~1.1us but
back-to-back descriptors on one queue pipeline.  So EVERYTHING - input
loads in deadline order, then output stores - rides the SP (sync) queue,
with sample 0 split [w taps0-4 | x rows0-17 | rest] so its first conv
block can start right as the warmup matmuls drain (~10.6us); slice-level
tile deps gate each matmul on exactly the bytes it reads.  The PE warmup
covers engine start -> first data and MUST stay contiguous with the real
matmul stream: an idle gap over ~1us before the HAM activity window fires
(~3.4us of busy) restarts the 1.2->2.4GHz clock ramp and costs ~4us
(measured in v2).

The bias-add + PSUM->SBUF eviction runs on the otherwise-idle Vector engine
(tensor_scalar_add); outputs are staged fp16 (host converts back; ~2.8e-4
added rel err in quadrature) and the last sample stores per 16-row block so
the final store is 128KB right behind its sibling on the pipelined queue.
"""

import os
import numpy as np

import concourse.bacc as bacc
import concourse.tile as tile
from concourse import mybir
from concourse.bass_utils import run_bass_kernel_spmd

N_CORES = 8
B_I, B_J, C, H, W = 32, 1, 128, 32, 32
OC, KH, KW = 128, 3, 3
S = B_I // N_CORES            # samples per core
HP, WP = H + 2, W + 2         # padded image
NTAP = KH * KW                # 9
W_COLS = NTAP * OC            # weight columns per sample (1152)
X_COLS = HP * WP              # padded image columns (1156)
TOT_COLS = W_COLS + X_COLS + 2  # + 2 cols holding the fp32 bias bit pattern
X0T_ROWS = 18                 # padded rows needed by sample 0's first block

_DT_TABLE = {
    "fp32": (mybir.dt.float32, np.float32),
    "fp32r": (mybir.dt.float32r, np.float32),
    "fp16": (mybir.dt.float16, np.float16),
    "bf16": (mybir.dt.bfloat16, None),  # np dtype filled lazily below
}

# Matmul operand dtype (walrus requires x and w to be both 16-bit or both
# 32-bit). Default fp16: 1 PE cycle/row with fast weight load, measured rel
# err ~2.9e-4 vs the fp32 reference. fp32r gives ~1.5e-4 at ~+15% time;
# fp32 gives ~3e-7 at ~2.5x time.
_MM_DT_NAME = os.environ.get("CONV_MM_DTYPE", "fp16")
MM_DT, MM_NP = _DT_TABLE[_MM_DT_NAME]
if MM_NP is None:
    import ml_dtypes

    MM_NP = ml_dtypes.bfloat16
X_DT = W_DT = MM_DT
X_NP = W_NP = MM_NP

OUT_DT, OUT_NP = mybir.dt.float16, np.float16

# Garbage matmuls that keep the PE busy from engine start until sample 0's
# first-block data lands (~2.7us later), so the HAM clock-gate ramp overlaps
# the input DMA.  The count is tuned to the measured DMA arrival; the real
# matmul stream must follow with no significant idle gap (see module doc).
# Must exceed the HAM 4096-cycle activity window (~3.4us busy = 32 matmuls)
# regardless of phase: shorter warmups + an idle gap before data arrives can
# leave the window unfired and the ramp resets (measured +2us or worse).
WARMUP_MMS = int(os.environ.get("CONV_WARMUP_MMS", "32"))


# Row blocks per sample: plain [16, 16] except the last sample, whose tail
# blocks shrink so the end-of-kernel bias-add + final store are small.
def _blocks(s):
    if s == S - 1:
        return [(0, 16), (16, 12), (28, 4)]
    return [(0, 16), (16, 16)]


# test.py hooks: set TRACE=True before calling kernel() to profile; the
# BassKernelResults of the last run lands in LAST_RESULTS.
TRACE = False
TRACE_KW = {}
LAST_RESULTS = None

_NC_CACHE = None


def _build_nc():
    f32 = mybir.dt.float32
    nc = bacc.Bacc()
    xw_d = nc.declare_dram_parameter("xw", [S, C, TOT_COLS], MM_DT, isOutput=False)
    o_d = nc.declare_dram_parameter("o", [S, OC, H, W], OUT_DT, isOutput=True)

    with tile.TileContext(nc, pool_alloc_mode="queue") as tc:
        with (
            tc.tile_pool(name="ins", bufs=1) as ins_pool,
            tc.tile_pool(name="outs", bufs=1) as outs_pool,
            tc.tile_pool(name="psum", bufs=8, space="PSUM") as psum_pool,
        ):
            wu_x = ins_pool.tile([C, OC], W_DT, tag="warmup", name="warmup")
            nc.gpsimd.memset(wu_x[:], 0.0)
            wu_ps = psum_pool.tile([OC, 16, W], f32, name="wu_ps", tag="ps")
            for _ in range(WARMUP_MMS):
                nc.tensor.matmul(
                    wu_ps[:, :4, :], wu_x[:], wu_x[:], start=True, stop=True
                )

            xw_ts = [
                ins_pool.tile([C, TOT_COLS], MM_DT, tag=f"xw{s}", name=f"xw{s}")
                for s in range(S)
            ]
            wts = [t[:, :W_COLS] for t in xw_ts]
            xvs = [
                t[:, W_COLS : W_COLS + X_COLS].rearrange("p (h w) -> p h w", w=WP)
                for t in xw_ts
            ]
            biases = [t[:, W_COLS + X_COLS :].bitcast(f32) for t in xw_ts]

            # SP queue in deadline order.  Sample 0's top image rows come
            # first (every tap's matmul reads them), then its weights in
            # 3-tap chunks so block 0's tap-0 matmul starts as soon as x0t
            # + the first chunk land, with later chunks streaming in faster
            # (~0.12us/tap) than the PE consumes them (~0.22us/tap).  Then
            # xw1..3 land ahead of their samples.  Sample 0's bottom image
            # rows + bias (deadline ~12.4us) ride the otherwise-idle ACT
            # queue - slow (~3.2us ramp, ~0.1 B/ns) but comfortably in time,
            # and they shorten the SP queue so xw1 clears sample 1's
            # deadline.
            x0_mid = W_COLS + X0T_ROWS * WP
            nc.sync.dma_start(xw_ts[0][:, W_COLS:x0_mid], xw_d[0][:, W_COLS:x0_mid])
            for c0, c1 in ((0, 3 * OC), (3 * OC, 6 * OC), (6 * OC, W_COLS)):
                nc.sync.dma_start(xw_ts[0][:, c0:c1], xw_d[0][:, c0:c1])
            nc.scalar.dma_start(xw_ts[0][:, x0_mid:], xw_d[0][:, x0_mid:])
            nc.sync.dma_start(xw_ts[1][:], xw_d[1])
            nc.sync.dma_start(xw_ts[2][:], xw_d[2])
            nc.sync.dma_start(xw_ts[3][:], xw_d[3])

            def conv_block(s, row0, nrows, ps_name):
                """One accumulation group: output rows [row0, row0+nrows)."""
                ps = psum_pool.tile([OC, 16, W], f32, name=ps_name, tag="ps")
                for t in range(NTAP):
                    kh, kw = divmod(t, KW)
                    rhs = xvs[s][:, row0 + kh : row0 + kh + nrows, kw : kw + W]
                    lhsT = wts[s][:, t * OC : (t + 1) * OC]
                    nc.tensor.matmul(
                        ps[:, :nrows, :],
                        lhsT,
                        rhs,
                        start=(t == 0),
                        stop=(t == NTAP - 1),
                    )
                return ps

            for s in range(S):
                out_t = outs_pool.tile(
                    [OC, H, W], OUT_DT, tag=f"out{s}", name=f"out{s}"
                )
                blocks = _blocks(s)
                for bi, (row0, nrows) in enumerate(blocks):
                    ps = conv_block(s, row0, nrows, f"ps{s}_{bi}")
                    nc.vector.tensor_scalar_add(
                        out_t[:, row0 : row0 + nrows, :],
                        ps[:, :nrows, :],
                        biases[s],
                    )
                    if s == S - 1:
                        # Stream the last sample per block so the final store
                        # is only 128KB and rides right behind its sibling.
                        nc.sync.dma_start(
                            o_d[s][:, row0 : row0 + nrows, :],
                            out_t[:, row0 : row0 + nrows, :],
                        )
                if s < S - 1:
                    nc.sync.dma_start(o_d[s], out_t[:])
    nc.compile()
    return nc


def _get_nc():
    global _NC_CACHE
    if _NC_CACHE is None:
        _NC_CACHE = _build_nc()
    return _NC_CACHE


def kernel(x: np.ndarray, weight: np.ndarray, bias: np.ndarray) -> np.ndarray:
    global LAST_RESULTS
    assert x.shape == (B_I, B_J, C, H, W)
    assert weight.shape == (B_I, OC, C, KH, KW)
    assert bias.shape == (B_I, B_J, OC)

    x = np.asarray(x, dtype=np.float32)
    weight = np.asarray(weight, dtype=np.float32)
    bias = np.asarray(bias, dtype=np.float32)

    # Host-side layout prep (part of sharding): zero-pad images, transpose
    # weights so each 3x3 tap is a contiguous [c, oc] stationary tile, and
    # append the per-sample fp32 bias bit pattern (partition oc) as 2 cols.
    xw = np.zeros((B_I, C, TOT_COLS), dtype=MM_NP)
    wt = np.ascontiguousarray(weight.transpose(0, 2, 3, 4, 1))  # [b_i, c, kh, kw, oc]
    xw[:, :, :W_COLS] = wt.reshape(B_I, C, W_COLS).astype(MM_NP)
    xpad = xw[:, :, W_COLS : W_COLS + X_COLS].reshape(B_I, C, HP, WP)
    xpad[:, :, 1 : 1 + H, 1 : 1 + W] = x[:, 0].astype(MM_NP)
    xw[:, :, W_COLS + X_COLS :].view(np.float32)[:, :, 0] = bias[:, 0, :]

    in_maps = []
    for core in range(N_CORES):
        sl = slice(core * S, (core + 1) * S)
        in_maps.append({"xw": np.ascontiguousarray(xw[sl])})

    nc = _get_nc()
    try:
        res = run_bass_kernel_spmd(
            nc, in_maps, core_ids=list(range(N_CORES)), trace=TRACE, **TRACE_KW
        )
    except Exception:
        # Transient NRT/device errors (e.g. NRT_EXEC_UNIT_UNRECOVERABLE after
        # heavy reuse) usually clear on retry; the work is idempotent.
        import time

        time.sleep(10)
        res = run_bass_kernel_spmd(
            nc, in_maps, core_ids=list(range(N_CORES)), trace=TRACE, **TRACE_KW
        )
    LAST_RESULTS = res

    out = np.concatenate([res.results[c]["o"] for c in range(N_CORES)], axis=0)
    return out.astype(np.float32).reshape(B_I, B_J, OC, H, W)

